# revision 1
# baseline (speedup 1.0000x reference)
"""Bass/Tile TRN2 kernel for nn_Attention_26388279067013.

Computes, for each batch row b:
    feat = enc @ We.T + dec @ Ws.T + cov[:,None] * Wc.sum(1) + b     [S, H]
    att  = tanh(feat) @ v_w                                          [S]
    att[s >= L_b] = -inf ; w = softmax(att) ; new_cov = cov + w
Returns (attention_weights [B,S], new_coverage [B,S]) both float32.

Sharding: data-parallel over B across 8 NeuronCores (4 rows each).
"""

import sys

sys.path.insert(0, "/opt/trn_rl_repo")

import numpy as np

import concourse.bacc as bacc
import concourse.tile as tile
import concourse.mybir as mybir
from concourse.bass_utils import run_bass_kernel_spmd

B, S, H, D = 32, 4096, 512, 256
N_CORES = 8
B_LOC = B // N_CORES          # 4 batch rows per core
F32 = mybir.dt.float32
F32R = mybir.dt.float32r
ALU = mybir.AluOpType
ACTF = mybir.ActivationFunctionType

N_K = H // 128                # 4 contraction tiles
N_STILE = S // 128            # 32 psum s-tiles per batch
N_CHUNK = S // 512            # 8 dma chunks per batch
NEG_BIG = -30000.0            # exp(x - 30000) == 0.0 exactly in f32


def r32(ap):
    return ap.bitcast(F32R)


def build_kernel():
    nc = bacc.Bacc("TRN2", debug=False, num_devices=N_CORES)

    # ---- dram I/O (per core) ----
    encT = nc.dram_tensor("encT", [B_LOC, H, S], F32, kind="ExternalInput").ap()
    cov = nc.dram_tensor("cov", [B_LOC, 32, 128], F32, kind="ExternalInput").ap()
    dec_cols = nc.dram_tensor("dec_cols", [B_LOC, 128, 2], F32, kind="ExternalInput").ap()
    lens = nc.dram_tensor("lens", [B_LOC, 1], F32, kind="ExternalInput").ap()
    WeT = nc.dram_tensor("WeT", [H, H], F32, kind="ExternalInput").ap()
    WcT = nc.dram_tensor("WcT", [H, H], F32, kind="ExternalInput").ap()
    WsT = nc.dram_tensor("WsT", [D, H], F32, kind="ExternalInput").ap()
    b_row = nc.dram_tensor("b_row", [1, H], F32, kind="ExternalInput").ap()
    v_row = nc.dram_tensor("v_row", [1, H], F32, kind="ExternalInput").ap()
    iota_d = nc.dram_tensor("iota_pm", [128, 32], F32, kind="ExternalInput").ap()
    ident_d = nc.dram_tensor("ident", [128, 128], F32, kind="ExternalInput").ap()
    ones_row = nc.dram_tensor("ones_row", [1, S], F32, kind="ExternalInput").ap()
    out_w = nc.dram_tensor("out_w", [B_LOC, 32, 128], F32, kind="ExternalOutput").ap()
    out_c = nc.dram_tensor("out_c", [B_LOC, 32, 128], F32, kind="ExternalOutput").ap()

    with tile.TileContext(nc) as tc:
        with (
            tc.tile_pool(name="persist", bufs=1) as pp,
            tc.tile_pool(name="enc", bufs=12) as encp,
            tc.tile_pool(name="x", bufs=3) as xp,
            tc.tile_pool(name="scratch", bufs=2) as scrp,
            tc.tile_pool(name="small", bufs=4) as smp,
            tc.tile_pool(name="batch", bufs=3) as bp,
            tc.tile_pool(name="psum", bufs=2, space="PSUM") as psp,
            tc.tile_pool(name="psum_misc", bufs=4, space="PSUM") as psm,
        ):
            # ---- one-time setup ----
            wet = []
            for k in range(N_K):
                t = pp.tile([128, H], F32R, tag=f"wet{k}")
                nc.scalar.dma_start(t[:], r32(WeT[k * 128:(k + 1) * 128, :]))
                wet.append(t)
            wst = []
            for k in range(D // 128):
                t = pp.tile([128, H], F32, tag=f"wst{k}")
                nc.scalar.dma_start(t[:], WsT[k * 128:(k + 1) * 128, :])
                wst.append(t)
            brow_sb = pp.tile([1, H], F32, tag="brow")
            nc.scalar.dma_start(brow_sb[:], b_row[:, :])
            vrow_sb = pp.tile([1, H], F32, tag="vrow")
            nc.scalar.dma_start(vrow_sb[:], v_row[:, :])
            ones_k1 = pp.tile([1, 128], F32, tag="ones_k1")
            nc.vector.memset(ones_k1[:], 1.0)
            ones_col = pp.tile([128, 1], F32, tag="ones_col")
            nc.vector.memset(ones_col[:], 1.0)

            # dep-free matmul burst: trips the PE HAM to K=8/8 (~2.4 GHz)
            # before the real stream arrives, instead of ~40us into it.
            warm_f = pp.tile([128, 512], F32, tag="warm_f")
            nc.vector.memset(warm_f[:], 0.5)
            warm = pp.tile([128, 512], F32R, tag="warm")
            nc.scalar.dma_start(warm[:], r32(warm_f[:]))
            for wi in range(20):
                ps_w = psm.tile([128, 512], F32, tag="mpsum")
                nc.tensor.matmul(ps_w[:], warm[:, 0:128], warm[:],
                                 start=True, stop=True)

            # wc_sum[o] = sum_h WcT[h, o]  -> [1, 512]
            ps_wc = psm.tile([1, H], F32, tag="mpsum")
            for k in range(N_K):
                t = scrp.tile([128, H], F32, tag="wct")
                nc.scalar.dma_start(t[:], WcT[k * 128:(k + 1) * 128, :])
                nc.tensor.matmul(ps_wc[:], ones_col[:], t[:],
                                 start=(k == 0), stop=(k == N_K - 1))
            wc_row = pp.tile([1, H], F32, tag="wc_row")
            nc.scalar.copy(wc_row[:], ps_wc[:])

            # v_bcast[p, o] = v_w[o]
            ps_vb = psm.tile([128, H], F32, tag="mpsum")
            nc.tensor.matmul(ps_vb[:], ones_k1[:], vrow_sb[:],
                             start=True, stop=True)
            v_bcast = pp.tile([128, H], F32, tag="v_bcast")
            nc.scalar.copy(v_bcast[:], ps_vb[:])

            iota_sb = pp.tile([128, 32], F32, tag="iota")
            ident_sb = pp.tile([128, 128], F32, tag="ident")

            # ---- per batch, software-pipelined ----
            # prep(b) builds per-batch small operands; heavy(b) is the matmul
            # stream; softmax(b) is emitted in the middle of heavy(b+1) so the
            # PE never drains at a batch boundary (keeps HAM warm).
            state = {}

            def emit_prep(b):
                dc = smp.tile([128, 2], F32, tag="dc")
                nc.scalar.dma_start(dc[:], dec_cols[b, :, :])
                ps_row = psm.tile([1, H], F32, tag="mpsum")
                for j in range(D // 128):
                    nc.tensor.matmul(ps_row[:], dc[:, j:j + 1], wst[j][:],
                                     start=(j == 0), stop=(j == 1))
                aug_st = bp.tile([2, H], F32, tag="aug_st")
                nc.vector.tensor_tensor(aug_st[0:1, :], ps_row[:], brow_sb[:], ALU.add)
                nc.scalar.dma_start(aug_st[1:2, :], wc_row[:])
                aug_rhs = bp.tile([2, H], F32R, tag="aug_rhs")
                nc.scalar.dma_start(aug_rhs[:], r32(aug_st[:]))

                cov_aug = bp.tile([2, S], F32R, tag="cov_aug")
                nc.scalar.dma_start(cov_aug[0:1, :], r32(ones_row[:, :]))
                nc.scalar.dma_start(
                    cov_aug[1:2, :],
                    r32(cov[b:b + 1].rearrange("c a b -> c (a b)")),
                )

                len_sb = smp.tile([1, 1], F32, tag="len_sb")
                nc.scalar.dma_start(len_sb[:], lens[b:b + 1, :])
                ps_l = psm.tile([128, 1], F32, tag="mpsum")
                nc.tensor.matmul(ps_l[:], ones_k1[:], len_sb[:],
                                 start=True, stop=True)
                l_col = smp.tile([128, 1], F32, tag="l_col")
                nc.scalar.copy(l_col[:], ps_l[:])

                att_pm = bp.tile([128, 32], F32, tag="att_pm")
                state[b] = dict(aug_rhs=aug_rhs, cov_aug=cov_aug,
                                l_col=l_col, att_pm=att_pm)

            def emit_heavy_chunk(b, c):
                st8 = state[b]
                ek = []
                for k in range(N_K):
                    t = encp.tile([128, 512], F32R, tag="enc")
                    nc.sync.dma_start(
                        t[:], r32(encT[b, k * 128:(k + 1) * 128, c * 512:(c + 1) * 512]))
                    ek.append(t)
                for t2 in range(2):
                    ps = psp.tile([128, 1024], F32, tag="feat")
                    for half in range(2):
                        st = 4 * c + 2 * t2 + half
                        scol = (2 * t2 + half) * 128
                        dst = ps[:, half * 512:(half + 1) * 512]
                        for k in range(N_K):
                            nc.tensor.matmul(
                                dst, ek[k][:, scol:scol + 128], wet[k][:],
                                start=(k == 0), stop=False)
                        nc.tensor.matmul(
                            dst, st8["cov_aug"][:, st * 128:(st + 1) * 128],
                            st8["aug_rhs"][:], start=False, stop=True)
                    x = xp.tile([128, 1024], F32, tag="x")
                    nc.scalar.activation(x[:], ps[:], ACTF.Tanh)
                    for half in range(2):
                        st = 4 * c + 2 * t2 + half
                        scr = scrp.tile([128, 512], F32, tag="vscr")
                        nc.vector.scalar_tensor_tensor(
                            scr[:], x[:, half * 512:(half + 1) * 512],
                            1.0, v_bcast[:], ALU.bypass, ALU.mult,
                            accum_out=st8["att_pm"][:, st:st + 1])

            def emit_softmax(b):
                st8 = state.pop(b)
                att_pm, l_col = st8["att_pm"], st8["l_col"]
                pad01 = bp.tile([128, 32], F32, tag="pad01")
                nc.vector.tensor_scalar(pad01[:], iota_sb[:], l_col[:], None, ALU.is_ge)
                att_m = bp.tile([128, 32], F32, tag="att_m")
                nc.vector.scalar_tensor_tensor(
                    att_m[:], pad01[:], NEG_BIG, att_pm[:], ALU.mult, ALU.add)
                exp_pm = bp.tile([128, 32], F32, tag="exp_pm")
                rowsum = smp.tile([128, 1], F32, tag="rowsum")
                nc.scalar.activation(exp_pm[:], att_m[:], ACTF.Exp, accum_out=rowsum[:])
                ps_d = psm.tile([1, 1], F32, tag="mpsum")
                nc.tensor.matmul(ps_d[:], rowsum[:], ones_col[:],
                                 start=True, stop=True)
                rinv = smp.tile([1, 1], F32, tag="rinv")
                nc.vector.reciprocal(rinv[:], ps_d[:])
                ps_r = psm.tile([128, 1], F32, tag="mpsum")
                nc.tensor.matmul(ps_r[:], ones_k1[:], rinv[:],
                                 start=True, stop=True)
                rinv_col = smp.tile([128, 1], F32, tag="rinv_col")
                nc.scalar.copy(rinv_col[:], ps_r[:])
                w_pm = bp.tile([128, 32], F32, tag="w_pm")
                nc.vector.tensor_scalar(w_pm[:], exp_pm[:], rinv_col[:], None, ALU.mult)

                ps_t = psm.tile([32, 128], F32, tag="mpsum")
                nc.tensor.transpose(ps_t[:], w_pm[:], ident_sb[:])
                covT = bp.tile([32, 128], F32, tag="covT")
                nc.scalar.dma_start(covT[:], cov[b, :, :])
                w_sb = bp.tile([32, 128], F32, tag="w_sb")
                nc.scalar.copy(w_sb[:], ps_t[:])
                ncov = bp.tile([32, 128], F32, tag="ncov")
                nc.vector.tensor_tensor(ncov[:], ps_t[:], covT[:], ALU.add)
                nc.scalar.dma_start(out_w[b, :, :], w_sb[:])
                nc.scalar.dma_start(out_c[b, :, :], ncov[:])

            emit_prep(0)
            emit_prep(1)
            nc.scalar.dma_start(iota_sb[:], iota_d[:, :])
            nc.scalar.dma_start(ident_sb[:], ident_d[:, :])
            for b in range(B_LOC):
                for c in range(N_CHUNK):
                    emit_heavy_chunk(b, c)
                    if c == 2 and b >= 1:
                        emit_softmax(b - 1)
                    if c == 5 and b + 2 < B_LOC:
                        emit_prep(b + 2)
            emit_softmax(B_LOC - 1)

    nc.compile()
    return nc


_NC_CACHE = {}


def _get_nc():
    if "nc" not in _NC_CACHE:
        _NC_CACHE["nc"] = build_kernel()
    return _NC_CACHE["nc"]


def make_in_maps(dec_input, enc_output, coverage_vector, text_lengths, W, b, v_w, v_b):
    dec_input = np.asarray(dec_input, np.float32)
    enc_output = np.ascontiguousarray(np.asarray(enc_output, np.float32))
    coverage_vector = np.asarray(coverage_vector, np.float32)
    lens_f = np.asarray(text_lengths).astype(np.float32)
    W = np.asarray(W, np.float32)
    b = np.asarray(b, np.float32)
    v_w = np.asarray(v_w, np.float32)

    WeT = np.ascontiguousarray(W[:, :H].T)            # [H, H]
    WsT = np.ascontiguousarray(W[:, H:H + D].T)       # [D, H]
    WcT = np.ascontiguousarray(W[:, H + D:].T)        # [H, H]
    b_rw = np.ascontiguousarray(b[None, :])
    v_rw = np.ascontiguousarray(v_w[None, :])
    iota_pm = (np.arange(32)[None, :] * 128 + np.arange(128)[:, None]).astype(np.float32)
    ident = np.eye(128, dtype=np.float32)

    in_maps = []
    for core in range(N_CORES):
        lo = core * B_LOC
        hi = lo + B_LOC
        encT = np.ascontiguousarray(enc_output[lo:hi].transpose(0, 2, 1))  # [B_LOC, H, S]
        covc = np.ascontiguousarray(coverage_vector[lo:hi].reshape(B_LOC, 32, 128))
        decc = np.ascontiguousarray(
            dec_input[lo:hi, 0, :].reshape(B_LOC, 2, 128).transpose(0, 2, 1))
        in_maps.append({
            "encT": encT,
            "cov": covc,
            "dec_cols": decc,
            "lens": np.ascontiguousarray(lens_f[lo:hi].reshape(B_LOC, 1)),
            "WeT": WeT, "WcT": WcT, "WsT": WsT,
            "b_row": b_rw, "v_row": v_rw,
            "iota_pm": iota_pm, "ident": ident,
            "ones_row": np.ones((1, S), np.float32),
        })
    return in_maps


def kernel(dec_input, enc_output, coverage_vector, text_lengths, W, b, v_w, v_b,
           _trace=False):
    nc = _get_nc()
    in_maps = make_in_maps(dec_input, enc_output, coverage_vector, text_lengths,
                           W, b, v_w, v_b)
    res = run_bass_kernel_spmd(nc, in_maps, list(range(N_CORES)), trace=_trace)
    w = np.concatenate([r["out_w"].reshape(B_LOC, S) for r in res.results], axis=0)
    c = np.concatenate([r["out_c"].reshape(B_LOC, S) for r in res.results], axis=0)
    if _trace:
        kernel.last_result = res
    return w, c



# revision 2
# speedup vs baseline: 1.1748x; 1.1748x over previous
"""Bass/Tile TRN2 kernel for nn_Attention_26388279067013.

Computes, for each batch row b:
    feat = enc @ We.T + dec @ Ws.T + cov[:,None] * Wc.sum(1) + b     [S, H]
    att  = tanh(feat) @ v_w                                          [S]
    att[s >= L_b] = -inf ; w = softmax(att) ; new_cov = cov + w
Returns (attention_weights [B,S], new_coverage [B,S]) both float32.

Sharding: data-parallel over B across 8 NeuronCores (4 rows each).

The matmul stream runs in bf16 (enc/We/aug quantized host-side or
on-chip): the PE sustains 1 col/cycle with 2-byte moving operands vs
~2 col/cycle for fp32, and enc DMA bytes halve. PSUM accumulation and
the whole softmax path stay fp32, so the only error is input
quantization (~3e-3 rel, well under the 2e-2 gate).
"""

import sys

sys.path.insert(0, "/opt/trn_rl_repo")

import ml_dtypes
import numpy as np

import concourse.bacc as bacc
import concourse.tile as tile
import concourse.mybir as mybir
from concourse.bass_utils import run_bass_kernel_spmd

B, S, H, D = 32, 4096, 512, 256
N_CORES = 8
B_LOC = B // N_CORES          # 4 batch rows per core
F32 = mybir.dt.float32
BF16 = mybir.dt.bfloat16
ALU = mybir.AluOpType
ACTF = mybir.ActivationFunctionType
BF16_NP = ml_dtypes.bfloat16

N_K = H // 128                # 4 contraction tiles
N_STILE = S // 128            # 32 psum s-tiles per batch
N_CHUNK = S // 1024           # 4 dma chunks per batch (2KB/partition lines)
NEG_BIG = -30000.0            # exp(x - 30000) == 0.0 exactly in f32


def build_kernel():
    nc = bacc.Bacc("TRN2", debug=False, num_devices=N_CORES)

    # ---- dram I/O (per core) ----
    encT = nc.dram_tensor("encT", [B_LOC, H, S], BF16, kind="ExternalInput").ap()
    cov = nc.dram_tensor("cov", [B_LOC, 32, 128], F32, kind="ExternalInput").ap()
    cov16 = nc.dram_tensor("cov16", [B_LOC, 1, S], BF16, kind="ExternalInput").ap()
    dec_cols = nc.dram_tensor("dec_cols", [B_LOC, 128, 2], BF16, kind="ExternalInput").ap()
    lens = nc.dram_tensor("lens", [B_LOC, 1], F32, kind="ExternalInput").ap()
    WeT = nc.dram_tensor("WeT", [H, H], BF16, kind="ExternalInput").ap()
    WcT = nc.dram_tensor("WcT", [H, H], F32, kind="ExternalInput").ap()
    WsT = nc.dram_tensor("WsT", [D, H], BF16, kind="ExternalInput").ap()
    b_row = nc.dram_tensor("b_row", [1, H], F32, kind="ExternalInput").ap()
    v_row = nc.dram_tensor("v_row", [1, H], F32, kind="ExternalInput").ap()
    iota_d = nc.dram_tensor("iota_pm", [128, 32], F32, kind="ExternalInput").ap()
    ident_d = nc.dram_tensor("ident", [128, 128], F32, kind="ExternalInput").ap()
    ones_row = nc.dram_tensor("ones_row", [1, S], BF16, kind="ExternalInput").ap()
    out_w = nc.dram_tensor("out_w", [B_LOC, 32, 128], F32, kind="ExternalOutput").ap()
    out_c = nc.dram_tensor("out_c", [B_LOC, 32, 128], F32, kind="ExternalOutput").ap()

    with tile.TileContext(nc) as tc:
        with (
            tc.tile_pool(name="persist", bufs=1) as pp,
            tc.tile_pool(name="enc", bufs=10) as encp,
            tc.tile_pool(name="x", bufs=3) as xp,
            tc.tile_pool(name="scratch", bufs=2) as scrp,
            tc.tile_pool(name="small", bufs=4) as smp,
            tc.tile_pool(name="batch", bufs=3) as bp,
            tc.tile_pool(name="psum", bufs=2, space="PSUM") as psp,
            tc.tile_pool(name="psum_misc", bufs=4, space="PSUM") as psm,
        ):
            # ---- one-time setup ----
            wet = []
            for k in range(N_K):
                t = pp.tile([128, H], BF16, tag=f"wet{k}")
                nc.scalar.dma_start(t[:], WeT[k * 128:(k + 1) * 128, :])
                wet.append(t)
            wst = []
            for k in range(D // 128):
                t = pp.tile([128, H], BF16, tag=f"wst{k}")
                nc.scalar.dma_start(t[:], WsT[k * 128:(k + 1) * 128, :])
                wst.append(t)
            brow_sb = pp.tile([1, H], F32, tag="brow")
            nc.scalar.dma_start(brow_sb[:], b_row[:, :])
            vrow_sb = pp.tile([1, H], F32, tag="vrow")
            nc.scalar.dma_start(vrow_sb[:], v_row[:, :])
            ones_k1 = pp.tile([1, 128], F32, tag="ones_k1")
            nc.vector.memset(ones_k1[:], 1.0)
            ones_col = pp.tile([128, 1], F32, tag="ones_col")
            nc.vector.memset(ones_col[:], 1.0)

            # dep-free matmul burst: trips the PE HAM to K=8/8 (~2.4 GHz)
            # before the real stream arrives, instead of ~40us into it.
            warm = pp.tile([128, 512], BF16, tag="warm")
            nc.vector.memset(warm[:], 0.5)
            for wi in range(20):
                ps_w = psm.tile([128, 512], F32, tag="mpsum")
                nc.tensor.matmul(ps_w[:], warm[:, 0:128], warm[:],
                                 start=True, stop=True)

            # wc_sum[o] = sum_h WcT[h, o]  -> [1, 512]
            ps_wc = psm.tile([1, H], F32, tag="mpsum")
            for k in range(N_K):
                t = scrp.tile([128, H], F32, tag="wct")
                nc.scalar.dma_start(t[:], WcT[k * 128:(k + 1) * 128, :])
                nc.tensor.matmul(ps_wc[:], ones_col[:], t[:],
                                 start=(k == 0), stop=(k == N_K - 1))
            wc_row = pp.tile([1, H], F32, tag="wc_row")
            nc.scalar.copy(wc_row[:], ps_wc[:])

            # v_bcast[p, o] = v_w[o]   (bf16 copy for the DVE dot)
            ps_vb = psm.tile([128, H], F32, tag="mpsum")
            nc.tensor.matmul(ps_vb[:], ones_k1[:], vrow_sb[:],
                             start=True, stop=True)
            v_bcast = pp.tile([128, H], BF16, tag="v_bcast")
            nc.scalar.copy(v_bcast[:], ps_vb[:])

            iota_sb = pp.tile([128, 32], F32, tag="iota")
            ident_sb = pp.tile([128, 128], F32, tag="ident")

            # ---- per batch, software-pipelined ----
            # prep(b) builds per-batch small operands; heavy(b) is the matmul
            # stream; softmax(b) is emitted in the middle of heavy(b+1) so the
            # PE never drains at a batch boundary (keeps HAM warm).
            state = {}

            def emit_prep(b):
                dc = smp.tile([128, 2], BF16, tag="dc")
                nc.scalar.dma_start(dc[:], dec_cols[b, :, :])
                ps_row = psm.tile([1, H], F32, tag="mpsum")
                for j in range(D // 128):
                    nc.tensor.matmul(ps_row[:], dc[:, j:j + 1], wst[j][:],
                                     start=(j == 0), stop=(j == 1))
                aug_st = bp.tile([2, H], F32, tag="aug_st")
                nc.vector.tensor_tensor(aug_st[0:1, :], ps_row[:], brow_sb[:], ALU.add)
                nc.scalar.dma_start(aug_st[1:2, :], wc_row[:])
                aug_rhs = bp.tile([2, H], BF16, tag="aug_rhs")
                nc.scalar.copy(aug_rhs[:], aug_st[:])

                cov_aug = bp.tile([2, S], BF16, tag="cov_aug")
                nc.scalar.dma_start(cov_aug[0:1, :], ones_row[:, :])
                nc.scalar.dma_start(cov_aug[1:2, :], cov16[b, :, :])

                len_sb = smp.tile([1, 1], F32, tag="len_sb")
                nc.scalar.dma_start(len_sb[:], lens[b:b + 1, :])
                ps_l = psm.tile([128, 1], F32, tag="mpsum")
                nc.tensor.matmul(ps_l[:], ones_k1[:], len_sb[:],
                                 start=True, stop=True)
                l_col = smp.tile([128, 1], F32, tag="l_col")
                nc.scalar.copy(l_col[:], ps_l[:])

                att_pm = bp.tile([128, 32], F32, tag="att_pm")
                state[b] = dict(aug_rhs=aug_rhs, cov_aug=cov_aug,
                                l_col=l_col, att_pm=att_pm)

            def emit_heavy_chunk(b, c):
                st8 = state[b]
                ek = []
                for k in range(N_K):
                    t = encp.tile([128, 1024], BF16, tag="enc")
                    src = encT[b, k * 128:(k + 1) * 128, c * 1024:(c + 1) * 1024]
                    if k < 2:
                        nc.sync.dma_start(t[:], src)
                    else:
                        nc.gpsimd.dma_start(t[:], src)
                    ek.append(t)
                for t2 in range(4):
                    ps = psp.tile([128, 1024], F32, tag="feat")
                    for half in range(2):
                        st = 8 * c + 2 * t2 + half
                        scol = (2 * t2 + half) * 128
                        dst = ps[:, half * 512:(half + 1) * 512]
                        for k in range(N_K):
                            nc.tensor.matmul(
                                dst, ek[k][:, scol:scol + 128], wet[k][:],
                                start=(k == 0), stop=False)
                        nc.tensor.matmul(
                            dst, st8["cov_aug"][:, st * 128:(st + 1) * 128],
                            st8["aug_rhs"][:], start=False, stop=True)
                    x = xp.tile([128, 1024], BF16, tag="x")
                    nc.scalar.activation(x[:], ps[:], ACTF.Tanh)
                    for half in range(2):
                        st = 8 * c + 2 * t2 + half
                        scr = scrp.tile([128, 512], BF16, tag="vscr")
                        nc.vector.scalar_tensor_tensor(
                            scr[:], x[:, half * 512:(half + 1) * 512],
                            1.0, v_bcast[:], ALU.bypass, ALU.mult,
                            accum_out=st8["att_pm"][:, st:st + 1])

            def emit_softmax(b):
                st8 = state.pop(b)
                att_pm, l_col = st8["att_pm"], st8["l_col"]
                pad01 = bp.tile([128, 32], F32, tag="pad01")
                nc.vector.tensor_scalar(pad01[:], iota_sb[:], l_col[:], None, ALU.is_ge)
                att_m = bp.tile([128, 32], F32, tag="att_m")
                nc.vector.scalar_tensor_tensor(
                    att_m[:], pad01[:], NEG_BIG, att_pm[:], ALU.mult, ALU.add)
                exp_pm = bp.tile([128, 32], F32, tag="exp_pm")
                rowsum = smp.tile([128, 1], F32, tag="rowsum")
                nc.scalar.activation(exp_pm[:], att_m[:], ACTF.Exp, accum_out=rowsum[:])
                ps_d = psm.tile([1, 1], F32, tag="mpsum")
                nc.tensor.matmul(ps_d[:], rowsum[:], ones_col[:],
                                 start=True, stop=True)
                rinv = smp.tile([1, 1], F32, tag="rinv")
                nc.vector.reciprocal(rinv[:], ps_d[:])
                ps_r = psm.tile([128, 1], F32, tag="mpsum")
                nc.tensor.matmul(ps_r[:], ones_k1[:], rinv[:],
                                 start=True, stop=True)
                rinv_col = smp.tile([128, 1], F32, tag="rinv_col")
                nc.scalar.copy(rinv_col[:], ps_r[:])
                w_pm = bp.tile([128, 32], F32, tag="w_pm")
                nc.vector.tensor_scalar(w_pm[:], exp_pm[:], rinv_col[:], None, ALU.mult)

                ps_t = psm.tile([32, 128], F32, tag="mpsum")
                nc.tensor.transpose(ps_t[:], w_pm[:], ident_sb[:])
                covT = bp.tile([32, 128], F32, tag="covT")
                nc.scalar.dma_start(covT[:], cov[b, :, :])
                w_sb = bp.tile([32, 128], F32, tag="w_sb")
                nc.scalar.copy(w_sb[:], ps_t[:])
                ncov = bp.tile([32, 128], F32, tag="ncov")
                nc.vector.tensor_tensor(ncov[:], ps_t[:], covT[:], ALU.add)
                nc.scalar.dma_start(out_w[b, :, :], w_sb[:])
                nc.scalar.dma_start(out_c[b, :, :], ncov[:])

            emit_prep(0)
            emit_prep(1)
            nc.scalar.dma_start(iota_sb[:], iota_d[:, :])
            nc.scalar.dma_start(ident_sb[:], ident_d[:, :])
            for b in range(B_LOC):
                for c in range(N_CHUNK):
                    emit_heavy_chunk(b, c)
                    if c == 1 and b >= 1:
                        emit_softmax(b - 1)
                    if c == 2 and b + 2 < B_LOC:
                        emit_prep(b + 2)
            emit_softmax(B_LOC - 1)

    nc.compile()
    return nc


_NC_CACHE = {}


def _get_nc():
    if "nc" not in _NC_CACHE:
        _NC_CACHE["nc"] = build_kernel()
    return _NC_CACHE["nc"]


def make_in_maps(dec_input, enc_output, coverage_vector, text_lengths, W, b, v_w, v_b):
    dec_input = np.asarray(dec_input, np.float32)
    enc_output = np.asarray(enc_output, np.float32)
    coverage_vector = np.asarray(coverage_vector, np.float32)
    lens_f = np.asarray(text_lengths).astype(np.float32)
    W = np.asarray(W, np.float32)
    b = np.asarray(b, np.float32)
    v_w = np.asarray(v_w, np.float32)

    WeT = np.ascontiguousarray(W[:, :H].T.astype(BF16_NP))        # [H, H]
    WsT = np.ascontiguousarray(W[:, H:H + D].T.astype(BF16_NP))   # [D, H]
    WcT = np.ascontiguousarray(W[:, H + D:].T)                    # [H, H] f32
    b_rw = np.ascontiguousarray(b[None, :])
    v_rw = np.ascontiguousarray(v_w[None, :])
    iota_pm = (np.arange(32)[None, :] * 128 + np.arange(128)[:, None]).astype(np.float32)
    ident = np.eye(128, dtype=np.float32)

    in_maps = []
    for core in range(N_CORES):
        lo = core * B_LOC
        hi = lo + B_LOC
        encT = np.ascontiguousarray(
            enc_output[lo:hi].transpose(0, 2, 1).astype(BF16_NP))  # [B_LOC, H, S]
        covc = np.ascontiguousarray(coverage_vector[lo:hi].reshape(B_LOC, 32, 128))
        cov16c = np.ascontiguousarray(
            coverage_vector[lo:hi].reshape(B_LOC, 1, S).astype(BF16_NP))
        decc = np.ascontiguousarray(
            dec_input[lo:hi, 0, :].reshape(B_LOC, 2, 128).transpose(0, 2, 1)
            .astype(BF16_NP))
        in_maps.append({
            "encT": encT,
            "cov": covc,
            "cov16": cov16c,
            "dec_cols": decc,
            "lens": np.ascontiguousarray(lens_f[lo:hi].reshape(B_LOC, 1)),
            "WeT": WeT, "WcT": WcT, "WsT": WsT,
            "b_row": b_rw, "v_row": v_rw,
            "iota_pm": iota_pm, "ident": ident,
            "ones_row": np.ones((1, S), BF16_NP),
        })
    return in_maps


def kernel(dec_input, enc_output, coverage_vector, text_lengths, W, b, v_w, v_b,
           _trace=False):
    nc = _get_nc()
    in_maps = make_in_maps(dec_input, enc_output, coverage_vector, text_lengths,
                           W, b, v_w, v_b)
    res = run_bass_kernel_spmd(nc, in_maps, list(range(N_CORES)), trace=_trace)
    w = np.concatenate([r["out_w"].reshape(B_LOC, S) for r in res.results], axis=0)
    c = np.concatenate([r["out_c"].reshape(B_LOC, S) for r in res.results], axis=0)
    if _trace:
        kernel.last_result = res
    return w, c


# revision 4
# speedup vs baseline: 1.4871x; 1.2658x over previous
"""Bass/Tile TRN2 kernel for nn_Attention_26388279067013.

Computes, for each batch row b:
    feat = enc @ We.T + dec @ Ws.T + cov[:,None] * Wc.sum(1) + b     [S, H]
    att  = tanh(feat) @ v_w                                          [S]
    att[s >= L_b] = -inf ; w = softmax(att) ; new_cov = cov + w
Returns (attention_weights [B,S], new_coverage [B,S]) both float32.

Sharding: data-parallel over B across 8 NeuronCores (4 rows each).

The matmul stream runs in bf16 (inputs quantized host-side): the PE
sustains 1 col/cycle with 2-byte moving operands. The rank-2 term
(bias+dec row, cov*wc) is folded into a K=128 zero-padded matmul so
every PE instruction is a uniform full-width bf16 matmul -- no
row-group switches. All per-batch scalars (aug rows, lens, v
broadcast) are host-precomputed so the PE queue is warmup + pure
stream; DMA triggers are kept off the Scalar/ACT queue so tanh never
stalls behind them. PSUM accumulation and softmax stay fp32.
"""

import sys

sys.path.insert(0, "/opt/trn_rl_repo")

import ml_dtypes
import numpy as np

import concourse.bacc as bacc
import concourse.tile as tile
import concourse.mybir as mybir
from concourse.bass_utils import run_bass_kernel_spmd

B, S, H, D = 32, 4096, 512, 256
N_CORES = 8
B_LOC = B // N_CORES          # 4 batch rows per core
F32 = mybir.dt.float32
BF16 = mybir.dt.bfloat16
ALU = mybir.AluOpType
ACTF = mybir.ActivationFunctionType
BF16_NP = ml_dtypes.bfloat16

N_K = H // 128                # 4 contraction tiles
N_CHUNK = S // 1024           # 4 dma chunks per batch (2KB/partition lines)
NEG_BIG = -30000.0            # exp(x - 30000) == 0.0 exactly in f32
N_WARM = 24


def build_kernel():
    nc = bacc.Bacc("TRN2", debug=False, num_devices=N_CORES)

    # ---- dram I/O (per core) ----
    encT = nc.dram_tensor("encT", [B_LOC, H, S], BF16, kind="ExternalInput").ap()
    cov = nc.dram_tensor("cov", [B_LOC, 32, 128], F32, kind="ExternalInput").ap()
    cov16 = nc.dram_tensor("cov16", [B_LOC, 1, S], BF16, kind="ExternalInput").ap()
    aug2 = nc.dram_tensor("aug2", [B_LOC, 2, H], BF16, kind="ExternalInput").ap()
    lens_col = nc.dram_tensor("lens_col", [B_LOC, 128, 1], F32, kind="ExternalInput").ap()
    WeT = nc.dram_tensor("WeT", [H, H], BF16, kind="ExternalInput").ap()
    v_bc_d = nc.dram_tensor("v_bc", [128, H], BF16, kind="ExternalInput").ap()
    iota_d = nc.dram_tensor("iota_pm", [128, 32], F32, kind="ExternalInput").ap()
    ident_d = nc.dram_tensor("ident", [128, 128], F32, kind="ExternalInput").ap()
    ones_row = nc.dram_tensor("ones_row", [1, S], BF16, kind="ExternalInput").ap()
    out_w = nc.dram_tensor("out_w", [B_LOC, 32, 128], F32, kind="ExternalOutput").ap()
    out_c = nc.dram_tensor("out_c", [B_LOC, 32, 128], F32, kind="ExternalOutput").ap()

    with tile.TileContext(nc) as tc:
        with (
            tc.tile_pool(name="persist", bufs=1) as pp,
            tc.tile_pool(name="enc", bufs=10) as encp,
            tc.tile_pool(name="x", bufs=3) as xp,
            tc.tile_pool(name="scratch", bufs=2) as scrp,
            tc.tile_pool(name="small", bufs=4) as smp,
            tc.tile_pool(name="batch", bufs=3) as bp,
            tc.tile_pool(name="psum", bufs=2, space="PSUM") as psp,
            tc.tile_pool(name="psum_misc", bufs=4, space="PSUM") as psm,
        ):
            # ---- one-time setup (no PE involvement except warmup) ----
            wet = []
            for k in range(N_K):
                t = pp.tile([128, H], BF16, tag=f"wet{k}")
                nc.sync.dma_start(t[:], WeT[k * 128:(k + 1) * 128, :])
                wet.append(t)
            v_bcast = pp.tile([128, H], BF16, tag="v_bcast")
            nc.gpsimd.dma_start(v_bcast[:], v_bc_d[:, :])
            ones_k1 = pp.tile([1, 128], F32, tag="ones_k1")
            nc.vector.memset(ones_k1[:], 1.0)
            ones_col = pp.tile([128, 1], F32, tag="ones_col")
            nc.vector.memset(ones_col[:], 1.0)
            iota_sb = pp.tile([128, 32], F32, tag="iota")
            nc.gpsimd.dma_start(iota_sb[:], iota_d[:, :])
            ident_sb = pp.tile([128, 128], F32, tag="ident")
            nc.gpsimd.dma_start(ident_sb[:], ident_d[:, :])

            # 3-way ring of per-batch rank-2 operands (written by prep(b),
            # read by heavy(b); 3 deep so prep(b+2) never clobbers live data)
            cov_pad = []
            aug128 = []
            for par in range(3):
                cp = pp.tile([128, S], BF16, tag=f"cov_pad{par}")
                nc.vector.memset(cp[:], 0.0)
                nc.sync.dma_start(cp[0:1, :], ones_row[:, :])
                cov_pad.append(cp)
                ag = pp.tile([128, H], BF16, tag=f"aug128_{par}")
                nc.vector.memset(ag[:], 0.0)
                aug128.append(ag)

            # dep-free matmul burst: keeps the PE HAM ramping toward K=8/8
            # (~2.4 GHz) while the first enc chunk DMAs land.
            warm = pp.tile([128, 512], BF16, tag="warm")
            nc.vector.memset(warm[:], 0.5)
            for wi in range(N_WARM):
                ps_w = psm.tile([128, 512], F32, tag="mpsum")
                nc.tensor.matmul(ps_w[:], warm[:, 0:128], warm[:],
                                 start=True, stop=True)

            # ---- per batch, software-pipelined ----
            state = {}

            def emit_prep(b):
                par = b % 3
                nc.sync.dma_start(cov_pad[par][1:2, :], cov16[b, :, :])
                nc.gpsimd.dma_start(aug128[par][0:2, :], aug2[b, :, :])
                l_col = smp.tile([128, 1], F32, tag="l_col")
                nc.sync.dma_start(l_col[:], lens_col[b, :, :])
                att_pm = bp.tile([128, 32], F32, tag="att_pm")
                state[b] = dict(l_col=l_col, att_pm=att_pm, par=par)

            def emit_heavy_chunk(b, c):
                st8 = state[b]
                par = st8["par"]
                ek = []
                for k in range(N_K):
                    t = encp.tile([128, 1024], BF16, tag="enc")
                    src = encT[b, k * 128:(k + 1) * 128, c * 1024:(c + 1) * 1024]
                    if k < 2:
                        nc.sync.dma_start(t[:], src)
                    else:
                        nc.gpsimd.dma_start(t[:], src)
                    ek.append(t)
                for t2 in range(4):
                    ps = psp.tile([128, 1024], F32, tag="feat")
                    for half in range(2):
                        st = 8 * c + 2 * t2 + half
                        scol = (2 * t2 + half) * 128
                        dst = ps[:, half * 512:(half + 1) * 512]
                        for k in range(N_K):
                            nc.tensor.matmul(
                                dst, ek[k][:, scol:scol + 128], wet[k][:],
                                start=(k == 0), stop=False)
                        nc.tensor.matmul(
                            dst, cov_pad[par][:, st * 128:(st + 1) * 128],
                            aug128[par][:], start=False, stop=True)
                    x = xp.tile([128, 1024], BF16, tag="x")
                    nc.scalar.activation(x[:], ps[:], ACTF.Tanh)
                    for half in range(2):
                        st = 8 * c + 2 * t2 + half
                        scr = scrp.tile([128, 512], BF16, tag="vscr")
                        nc.vector.scalar_tensor_tensor(
                            scr[:], x[:, half * 512:(half + 1) * 512],
                            1.0, v_bcast[:], ALU.bypass, ALU.mult,
                            accum_out=st8["att_pm"][:, st:st + 1])

            def emit_softmax(b):
                st8 = state.pop(b)
                att_pm, l_col = st8["att_pm"], st8["l_col"]
                pad01 = bp.tile([128, 32], F32, tag="pad01")
                nc.vector.tensor_scalar(pad01[:], iota_sb[:], l_col[:], None, ALU.is_ge)
                att_m = bp.tile([128, 32], F32, tag="att_m")
                nc.vector.scalar_tensor_tensor(
                    att_m[:], pad01[:], NEG_BIG, att_pm[:], ALU.mult, ALU.add)
                exp_pm = bp.tile([128, 32], F32, tag="exp_pm")
                rowsum = smp.tile([128, 1], F32, tag="rowsum")
                nc.scalar.activation(exp_pm[:], att_m[:], ACTF.Exp, accum_out=rowsum[:])
                ps_d = psm.tile([1, 1], F32, tag="mpsum")
                nc.tensor.matmul(ps_d[:], rowsum[:], ones_col[:],
                                 start=True, stop=True)
                rinv = smp.tile([1, 1], F32, tag="rinv")
                nc.vector.reciprocal(rinv[:], ps_d[:])
                ps_r = psm.tile([128, 1], F32, tag="mpsum")
                nc.tensor.matmul(ps_r[:], ones_k1[:], rinv[:],
                                 start=True, stop=True)
                rinv_col = smp.tile([128, 1], F32, tag="rinv_col")
                nc.scalar.copy(rinv_col[:], ps_r[:])
                w_pm = bp.tile([128, 32], F32, tag="w_pm")
                nc.vector.tensor_scalar(w_pm[:], exp_pm[:], rinv_col[:], None, ALU.mult)

                ps_t = psm.tile([32, 128], F32, tag="mpsum")
                nc.tensor.transpose(ps_t[:], w_pm[:], ident_sb[:])
                covT = bp.tile([32, 128], F32, tag="covT")
                nc.sync.dma_start(covT[:], cov[b, :, :])
                w_sb = bp.tile([32, 128], F32, tag="w_sb")
                nc.scalar.copy(w_sb[:], ps_t[:])
                ncov = bp.tile([32, 128], F32, tag="ncov")
                nc.vector.tensor_tensor(ncov[:], ps_t[:], covT[:], ALU.add)
                nc.sync.dma_start(out_w[b, :, :], w_sb[:])
                nc.sync.dma_start(out_c[b, :, :], ncov[:])

            emit_prep(0)
            emit_prep(1)
            for b in range(B_LOC):
                for c in range(N_CHUNK):
                    emit_heavy_chunk(b, c)
                    if c == 1 and b >= 1:
                        emit_softmax(b - 1)
                    if c == 2 and b + 2 < B_LOC:
                        emit_prep(b + 2)
            emit_softmax(B_LOC - 1)

    nc.compile()
    return nc


_NC_CACHE = {}


def _get_nc():
    if "nc" not in _NC_CACHE:
        _NC_CACHE["nc"] = build_kernel()
    return _NC_CACHE["nc"]


def make_in_maps(dec_input, enc_output, coverage_vector, text_lengths, W, b, v_w, v_b):
    dec_input = np.asarray(dec_input, np.float32)
    enc_output = np.asarray(enc_output, np.float32)
    coverage_vector = np.asarray(coverage_vector, np.float32)
    lens_f = np.asarray(text_lengths).astype(np.float32)
    W = np.asarray(W, np.float32)
    b = np.asarray(b, np.float32)
    v_w = np.asarray(v_w, np.float32)

    WeT = np.ascontiguousarray(W[:, :H].T.astype(BF16_NP))        # [H, H]
    Ws = W[:, H:H + D]                                            # [H, D]
    wc = W[:, H + D:].sum(axis=1)                                 # [H]
    # aug row per batch: dec @ Ws.T + b  (tiny; done host-side in f32)
    aug_rows = dec_input[:, 0, :] @ Ws.T + b[None, :]             # [B, H]
    v_bc = np.broadcast_to(v_w[None, :], (128, H)).astype(BF16_NP)
    iota_pm = (np.arange(32)[None, :] * 128 + np.arange(128)[:, None]).astype(np.float32)
    ident = np.eye(128, dtype=np.float32)

    in_maps = []
    for core in range(N_CORES):
        lo = core * B_LOC
        hi = lo + B_LOC
        encT = np.ascontiguousarray(
            enc_output[lo:hi].transpose(0, 2, 1).astype(BF16_NP))  # [B_LOC, H, S]
        covc = np.ascontiguousarray(coverage_vector[lo:hi].reshape(B_LOC, 32, 128))
        cov16c = np.ascontiguousarray(
            coverage_vector[lo:hi].reshape(B_LOC, 1, S).astype(BF16_NP))
        aug2c = np.ascontiguousarray(
            np.stack([aug_rows[lo:hi], np.broadcast_to(wc, (B_LOC, H))], axis=1)
            .astype(BF16_NP))                                      # [B_LOC, 2, H]
        lens_c = np.ascontiguousarray(
            np.broadcast_to(lens_f[lo:hi, None, None], (B_LOC, 128, 1))).copy()
        in_maps.append({
            "encT": encT,
            "cov": covc,
            "cov16": cov16c,
            "aug2": aug2c,
            "lens_col": lens_c,
            "WeT": WeT, "v_bc": np.ascontiguousarray(v_bc),
            "iota_pm": iota_pm, "ident": ident,
            "ones_row": np.ones((1, S), BF16_NP),
        })
    return in_maps


def kernel(dec_input, enc_output, coverage_vector, text_lengths, W, b, v_w, v_b,
           _trace=False):
    nc = _get_nc()
    in_maps = make_in_maps(dec_input, enc_output, coverage_vector, text_lengths,
                           W, b, v_w, v_b)
    res = run_bass_kernel_spmd(nc, in_maps, list(range(N_CORES)), trace=_trace)
    w = np.concatenate([r["out_w"].reshape(B_LOC, S) for r in res.results], axis=0)
    c = np.concatenate([r["out_c"].reshape(B_LOC, S) for r in res.results], axis=0)
    if _trace:
        kernel.last_result = res
    return w, c


# revision 7
# speedup vs baseline: 1.5348x; 1.0321x over previous
"""Bass/Tile TRN2 kernel for nn_Attention_26388279067013.

Computes, for each batch row b:
    feat = enc @ We.T + dec @ Ws.T + cov[:,None] * Wc.sum(1) + b     [S, H]
    att  = tanh(feat) @ v_w                                          [S]
    att[s >= L_b] = -inf ; w = softmax(att) ; new_cov = cov + w
Returns (attention_weights [B,S], new_coverage [B,S]) both float32.

Sharding: data-parallel over B across 8 NeuronCores (4 rows each).

The matmul stream runs in bf16 (inputs quantized host-side): the PE
sustains 1 col/cycle with 2-byte moving operands. The rank-2 term
(bias+dec row, cov*wc) is folded into a K=128 zero-padded matmul so
every PE instruction is a uniform full-width bf16 matmul -- no
row-group switches. All per-batch scalars (aug rows, lens, v
broadcast) are host-precomputed so the PE queue is warmup + pure
stream; DMA triggers are kept off the Scalar/ACT queue so tanh never
stalls behind them. PSUM accumulation and softmax stay fp32.
"""

import sys

sys.path.insert(0, "/opt/trn_rl_repo")

import ml_dtypes
import numpy as np

import concourse.bacc as bacc
import concourse.tile as tile
import concourse.mybir as mybir
from concourse.bass_utils import run_bass_kernel_spmd

B, S, H, D = 32, 4096, 512, 256
N_CORES = 8
B_LOC = B // N_CORES          # 4 batch rows per core
F32 = mybir.dt.float32
BF16 = mybir.dt.bfloat16
ALU = mybir.AluOpType
ACTF = mybir.ActivationFunctionType
BF16_NP = ml_dtypes.bfloat16

N_K = H // 128                # 4 contraction tiles
N_CHUNK = S // 1024           # 4 dma chunks per batch (2KB/partition lines)
NEG_BIG = -30000.0            # exp(x - 30000) == 0.0 exactly in f32
N_WARM = 16


def build_kernel():
    nc = bacc.Bacc("TRN2", debug=False, num_devices=N_CORES)

    # ---- dram I/O (per core) ----
    encT = nc.dram_tensor("encT", [B_LOC, H, S], BF16, kind="ExternalInput").ap()
    cov = nc.dram_tensor("cov", [B_LOC, 32, 128], F32, kind="ExternalInput").ap()
    cov16 = nc.dram_tensor("cov16", [B_LOC, 1, S], BF16, kind="ExternalInput").ap()
    aug2 = nc.dram_tensor("aug2", [B_LOC, 2, H], BF16, kind="ExternalInput").ap()
    lens_col = nc.dram_tensor("lens_col", [B_LOC, 128, 1], F32, kind="ExternalInput").ap()
    WeT = nc.dram_tensor("WeT", [H, H], BF16, kind="ExternalInput").ap()
    v_bc_d = nc.dram_tensor("v_bc", [128, H], BF16, kind="ExternalInput").ap()
    iota_d = nc.dram_tensor("iota_pm", [128, 32], F32, kind="ExternalInput").ap()
    ident_d = nc.dram_tensor("ident", [128, 128], F32, kind="ExternalInput").ap()
    ones_row = nc.dram_tensor("ones_row", [1, S], BF16, kind="ExternalInput").ap()
    out_w = nc.dram_tensor("out_w", [B_LOC, 32, 128], F32, kind="ExternalOutput").ap()
    out_c = nc.dram_tensor("out_c", [B_LOC, 32, 128], F32, kind="ExternalOutput").ap()

    with tile.TileContext(nc) as tc:
        with (
            tc.tile_pool(name="persist", bufs=1) as pp,
            tc.tile_pool(name="enc", bufs=10) as encp,
            tc.tile_pool(name="x", bufs=3) as xp,
            tc.tile_pool(name="scratch", bufs=2) as scrp,
            tc.tile_pool(name="small", bufs=4) as smp,
            tc.tile_pool(name="batch", bufs=3) as bp,
            tc.tile_pool(name="psum", bufs=2, space="PSUM") as psp,
            tc.tile_pool(name="psum_misc", bufs=4, space="PSUM") as psm,
        ):
            # ---- one-time setup (no PE involvement except warmup) ----
            # warm memset first on the vector queue so the PE warmup burst
            # starts ~1us in, not behind the big cov_pad ring memsets.
            warm = pp.tile([128, 512], BF16, tag="warm")
            nc.vector.memset(warm[:], 0.5)
            # dep-free matmul burst: keeps the PE HAM ramping toward K=8/8
            # (~2.4 GHz) while the first enc chunk DMAs land.
            for wi in range(N_WARM):
                ps_w = psm.tile([128, 512], F32, tag="mpsum")
                nc.tensor.matmul(ps_w[:], warm[:, 0:128], warm[:],
                                 start=True, stop=True)

            wet = []
            for k in range(N_K):
                t = pp.tile([128, H], BF16, tag=f"wet{k}")
                nc.sync.dma_start(t[:], WeT[k * 128:(k + 1) * 128, :])
                wet.append(t)
            v_bcast = pp.tile([128, H], BF16, tag="v_bcast")
            nc.gpsimd.dma_start(v_bcast[:], v_bc_d[:, :])
            ones_k1 = pp.tile([1, 128], F32, tag="ones_k1")
            nc.vector.memset(ones_k1[:], 1.0)
            ones_col = pp.tile([128, 1], F32, tag="ones_col")
            nc.vector.memset(ones_col[:], 1.0)
            iota_sb = pp.tile([128, 32], F32, tag="iota")
            nc.gpsimd.dma_start(iota_sb[:], iota_d[:, :])
            ident_sb = pp.tile([128, 128], F32, tag="ident")
            nc.gpsimd.dma_start(ident_sb[:], ident_d[:, :])

            # 3-way ring of per-batch rank-2 operands (written by prep(b),
            # read by heavy(b); 3 deep so prep(b+2) never clobbers live data)
            cov_pad = []
            aug128 = []
            for par in range(3):
                cp = pp.tile([128, S], BF16, tag=f"cov_pad{par}")
                nc.vector.memset(cp[:], 0.0)
                nc.sync.dma_start(cp[0:1, :], ones_row[:, :])
                cov_pad.append(cp)
                ag = pp.tile([128, H], BF16, tag=f"aug128_{par}")
                nc.vector.memset(ag[:], 0.0)
                aug128.append(ag)

            # ---- per batch, software-pipelined ----
            state = {}

            def emit_prep(b):
                par = b % 3
                nc.sync.dma_start(cov_pad[par][1:2, :], cov16[b, :, :])
                nc.gpsimd.dma_start(aug128[par][0:2, :], aug2[b, :, :])
                l_col = smp.tile([128, 1], F32, tag="l_col")
                nc.sync.dma_start(l_col[:], lens_col[b, :, :])
                att_pm = bp.tile([128, 32], F32, tag="att_pm")
                state[b] = dict(l_col=l_col, att_pm=att_pm, par=par)

            def emit_heavy_chunk(b, c):
                st8 = state[b]
                par = st8["par"]
                ek = []
                for k in range(N_K):
                    t = encp.tile([128, 1024], BF16, tag="enc")
                    src = encT[b, k * 128:(k + 1) * 128, c * 1024:(c + 1) * 1024]
                    if k < 2:
                        nc.sync.dma_start(t[:], src)
                    else:
                        nc.gpsimd.dma_start(t[:], src)
                    ek.append(t)
                for t2 in range(4):
                    ps = psp.tile([128, 1024], F32, tag="feat")
                    for half in range(2):
                        st = 8 * c + 2 * t2 + half
                        scol = (2 * t2 + half) * 128
                        dst = ps[:, half * 512:(half + 1) * 512]
                        for k in range(N_K):
                            nc.tensor.matmul(
                                dst, ek[k][:, scol:scol + 128], wet[k][:],
                                start=(k == 0), stop=False)
                        nc.tensor.matmul(
                            dst, cov_pad[par][:, st * 128:(st + 1) * 128],
                            aug128[par][:], start=False, stop=True)
                    x = xp.tile([128, 1024], BF16, tag="x")
                    nc.scalar.activation(x[:], ps[:], ACTF.Tanh)
                    for half in range(2):
                        st = 8 * c + 2 * t2 + half
                        scr = scrp.tile([128, 512], BF16, tag="vscr")
                        nc.vector.scalar_tensor_tensor(
                            scr[:], x[:, half * 512:(half + 1) * 512],
                            1.0, v_bcast[:], ALU.bypass, ALU.mult,
                            accum_out=st8["att_pm"][:, st:st + 1])

            def emit_softmax(b):
                # Two parallel chains after exp: (a) PE-transpose of exp,
                # (b) rowsum -> 1/sum -> broadcast to 32 partitions. They
                # join in two independent DVE ops (scale, scale+cov-add),
                # each followed directly by its output DMA — shortest
                # cross-engine hop chain for the last-batch tail.
                st8 = state.pop(b)
                att_pm, l_col = st8["att_pm"], st8["l_col"]
                pad01 = bp.tile([128, 32], F32, tag="pad01")
                nc.vector.tensor_scalar(pad01[:], iota_sb[:], l_col[:], None, ALU.is_ge)
                att_m = bp.tile([128, 32], F32, tag="att_m")
                nc.vector.scalar_tensor_tensor(
                    att_m[:], pad01[:], NEG_BIG, att_pm[:], ALU.mult, ALU.add)
                exp_pm = bp.tile([128, 32], F32, tag="exp_pm")
                rowsum = smp.tile([128, 1], F32, tag="rowsum")
                nc.scalar.activation(exp_pm[:], att_m[:], ACTF.Exp, accum_out=rowsum[:])
                covT = bp.tile([32, 128], F32, tag="covT")
                nc.sync.dma_start(covT[:], cov[b, :, :])

                ps_t = psm.tile([32, 128], F32, tag="mpsum")
                nc.tensor.transpose(ps_t[:], exp_pm[:], ident_sb[:])
                ps_d = psm.tile([1, 1], F32, tag="mpsum")
                nc.tensor.matmul(ps_d[:], rowsum[:], ones_col[:],
                                 start=True, stop=True)
                rinv = smp.tile([1, 1], F32, tag="rinv")
                nc.vector.reciprocal(rinv[:], ps_d[:])
                ps_r = psm.tile([32, 1], F32, tag="mpsum")
                nc.tensor.matmul(ps_r[:], ones_k1[:, 0:32], rinv[:],
                                 start=True, stop=True)
                rinv32 = smp.tile([32, 1], F32, tag="rinv32")
                nc.scalar.copy(rinv32[:], ps_r[:])

                w_sb = bp.tile([32, 128], F32, tag="w_sb")
                nc.vector.tensor_scalar(w_sb[:], ps_t[:], rinv32[:], None, ALU.mult)
                nc.sync.dma_start(out_w[b, :, :], w_sb[:])
                ncov = bp.tile([32, 128], F32, tag="ncov")
                nc.vector.scalar_tensor_tensor(
                    ncov[:], ps_t[:], rinv32[:], covT[:], ALU.mult, ALU.add)
                nc.sync.dma_start(out_c[b, :, :], ncov[:])

            emit_prep(0)
            emit_prep(1)
            for b in range(B_LOC):
                for c in range(N_CHUNK):
                    emit_heavy_chunk(b, c)
                    if c == 1 and b >= 1:
                        emit_softmax(b - 1)
                    if c == 2 and b + 2 < B_LOC:
                        emit_prep(b + 2)
            emit_softmax(B_LOC - 1)

    nc.compile()
    return nc


_NC_CACHE = {}


def _get_nc():
    if "nc" not in _NC_CACHE:
        _NC_CACHE["nc"] = build_kernel()
    return _NC_CACHE["nc"]


def make_in_maps(dec_input, enc_output, coverage_vector, text_lengths, W, b, v_w, v_b):
    dec_input = np.asarray(dec_input, np.float32)
    enc_output = np.asarray(enc_output, np.float32)
    coverage_vector = np.asarray(coverage_vector, np.float32)
    lens_f = np.asarray(text_lengths).astype(np.float32)
    W = np.asarray(W, np.float32)
    b = np.asarray(b, np.float32)
    v_w = np.asarray(v_w, np.float32)

    WeT = np.ascontiguousarray(W[:, :H].T.astype(BF16_NP))        # [H, H]
    Ws = W[:, H:H + D]                                            # [H, D]
    wc = W[:, H + D:].sum(axis=1)                                 # [H]
    # aug row per batch: dec @ Ws.T + b  (tiny; done host-side in f32)
    aug_rows = dec_input[:, 0, :] @ Ws.T + b[None, :]             # [B, H]
    v_bc = np.broadcast_to(v_w[None, :], (128, H)).astype(BF16_NP)
    iota_pm = (np.arange(32)[None, :] * 128 + np.arange(128)[:, None]).astype(np.float32)
    ident = np.eye(128, dtype=np.float32)

    in_maps = []
    for core in range(N_CORES):
        lo = core * B_LOC
        hi = lo + B_LOC
        encT = np.ascontiguousarray(
            enc_output[lo:hi].transpose(0, 2, 1).astype(BF16_NP))  # [B_LOC, H, S]
        covc = np.ascontiguousarray(coverage_vector[lo:hi].reshape(B_LOC, 32, 128))
        cov16c = np.ascontiguousarray(
            coverage_vector[lo:hi].reshape(B_LOC, 1, S).astype(BF16_NP))
        aug2c = np.ascontiguousarray(
            np.stack([aug_rows[lo:hi], np.broadcast_to(wc, (B_LOC, H))], axis=1)
            .astype(BF16_NP))                                      # [B_LOC, 2, H]
        lens_c = np.ascontiguousarray(
            np.broadcast_to(lens_f[lo:hi, None, None], (B_LOC, 128, 1))).copy()
        in_maps.append({
            "encT": encT,
            "cov": covc,
            "cov16": cov16c,
            "aug2": aug2c,
            "lens_col": lens_c,
            "WeT": WeT, "v_bc": np.ascontiguousarray(v_bc),
            "iota_pm": iota_pm, "ident": ident,
            "ones_row": np.ones((1, S), BF16_NP),
        })
    return in_maps


def kernel(dec_input, enc_output, coverage_vector, text_lengths, W, b, v_w, v_b,
           _trace=False):
    nc = _get_nc()
    in_maps = make_in_maps(dec_input, enc_output, coverage_vector, text_lengths,
                           W, b, v_w, v_b)
    res = run_bass_kernel_spmd(nc, in_maps, list(range(N_CORES)), trace=_trace)
    w = np.concatenate([r["out_w"].reshape(B_LOC, S) for r in res.results], axis=0)
    c = np.concatenate([r["out_c"].reshape(B_LOC, S) for r in res.results], axis=0)
    if _trace:
        kernel.last_result = res
    return w, c
